# revision 1
# baseline (speedup 1.0000x reference)
"""Correlation-layer kernel for Trainium2 (8 NeuronCores, data-parallel over batch).

Problem (per batch b):
    corr[k, m] = sum_c x[b, c, u, v] * y[b, c, i, j],  k = v*h+u, m = i*w+j
    out = relu(corr) / sqrt(sum_k relu(corr)^2 + eps)   (normalize over k per m)

Shapes: x, y = (8, 128, 48, 64) fp32 -> out (8, 3072, 48, 64) fp32.
Sharding: 1 batch per core. Per core it is a (3072x128)@(128x3072) matmul,
ReLU, and an L2 normalization over the 3072-channel dim.

Design (v3): natural layout (k on partitions, m on free dim), one matmul pass.
Per 512-wide m-chunk:
  - 24 f32r matmuls -> psum, emitted LOOKAHEAD chunks ahead so the PE stream
    stays dense (HAM stays un-throttled); relu evacuation to fp16 tiles that
    are kept in SBUF (ACT, with a few tiles on DVE for balance).
  - squares (fp16 2x mode, split DVE/ACT) feed a PE ones-matmul that
    accumulates sum-of-squares over the 24 k-tiles into a [1, 512] psum row.
  - recip chain: transpose ss row -> [128, 4] column (PE), sqrt (ACT) +
    reciprocal + Newton rsqrt refinement (DVE), transpose back, broadcast to
    [128, 512] fp16 via a rank-1 PE matmul.
  - output: multiply kept relu tiles by the broadcast recip (fp16 2x, split
    DVE/GpSimd) into one [128, 24*512] fp16 tile; a single SWDGE DMA casts
    fp16 -> fp32 while scattering into the output layout.
"""

import sys

sys.path.insert(0, "/opt/trn_rl_repo")

import numpy as np

_BUILD_CACHE = {}

B, C, H, W = 8, 128, 48, 64
K = W * H      # 3072 output channels, k = v*h+u
M = H * W      # 3072 spatial positions, m = i*w+j
CH = 512       # m-chunk width
NCH = M // CH  # 6 chunks
NKT = K // 128  # 24 k-tiles
EPS = 1e-6

LOOKAHEAD = 3   # how many chunks of A-matmuls run ahead
SQ_ACT_MOD = 0  # kt % SQ_ACT_MOD == 0 -> square on ACT (else DVE)
SC_GPS_MOD = 0  # kt % SC_GPS_MOD == 0 -> scale on GpSimd (else DVE)
RELU_DVE_MOD = 0  # kt % RELU_DVE_MOD == 0 -> relu evac on DVE (0 = never)


def build():
    from concourse import bacc, bass, mybir, tile

    F32 = mybir.dt.float32
    F32R = mybir.dt.float32r
    F16 = mybir.dt.float16
    AF = mybir.ActivationFunctionType
    OP = mybir.AluOpType

    nc = bacc.Bacc("TRN2", debug=False, target_bir_lowering=False)

    a_d = nc.dram_tensor("a", [C, K], F32R, kind="ExternalInput")
    b_d = nc.dram_tensor("b", [C, M], F32R, kind="ExternalInput")
    id_d = nc.dram_tensor("ident", [128, 128], F32, kind="ExternalInput")
    onc_d = nc.dram_tensor("onescol", [128, 1], F16, kind="ExternalInput")
    onr_d = nc.dram_tensor("onesrow", [1, 128], F32R, kind="ExternalInput")
    out_d = nc.dram_tensor("out", [K, M], F32, kind="ExternalOutput")
    junk_d = nc.dram_tensor("junkout", [128, CH], F32, kind="ExternalOutput")

    with tile.TileContext(nc) as tc:
        with (
            tc.tile_pool(name="pers", bufs=1) as pers,
            tc.tile_pool(name="rkeep", bufs=24 * (LOOKAHEAD + 1) + 2) as rkeep,
            tc.tile_pool(name="work", bufs=6) as work,
            tc.tile_pool(name="big", bufs=2) as bigp,
            tc.tile_pool(name="chain", bufs=2) as chain,
            tc.tile_pool(name="psA", bufs=5, space=bass.MemorySpace.PSUM) as psA,
            tc.tile_pool(name="psS", bufs=1, space=bass.MemorySpace.PSUM) as psS,
            tc.tile_pool(name="psJ", bufs=1, space=bass.MemorySpace.PSUM) as psJ,
            tc.tile_pool(name="psU", bufs=1, space=bass.MemorySpace.PSUM) as psU,
        ):
            a_t = pers.tile([C, K], F32R)
            b_t = pers.tile([C, M], F32R)
            id_t = pers.tile([128, 128], F32)
            onc_t = pers.tile([128, 1], F16)
            onr_t = pers.tile([1, 128], F32R)
            nc.sync.dma_start(a_t[:], a_d[:])
            nc.sync.dma_start(b_t[:], b_d[:])
            nc.sync.dma_start(id_t[:], id_d[:])
            nc.sync.dma_start(onc_t[:], onc_d[:])
            nc.sync.dma_start(onr_t[:], onr_d[:])

            sqs = {}
            relus = {}
            ss_rows = {}
            junk_ps = psJ.tile([128, CH], F32, tag="junk")

            def jmm(n=1):
                # dead matmuls that keep the PE's HAM activity window busy so
                # the clock gate stays at 2.4 GHz (real stream is evac-paced)
                for _ in range(n):
                    nc.tensor.matmul(
                        junk_ps[:], a_t[:, 0:128], b_t[:, 0:CH],
                        start=True, stop=True, skip_group_check=True,
                    )

            def emit_A_and_ones(c, cprev):
                """Interleave chunk c's matmuls+relu+square with chunk
                cprev's ones-matmul ss reduction so the PE stream stays dense
                and ACT/DVE are fed from the first tile."""
                sq_prev = sqs.pop(cprev) if cprev is not None else None
                if cprev is not None:
                    ss_ps = psS.tile([1, CH], F32, tag="ss")
                    ss_rows[cprev] = ss_ps
                sq = []
                rl = []
                for kt in range(NKT):
                    if c is not None:
                        m0 = c * CH
                        pA = psA.tile([128, CH], F32, tag="pA")
                        nc.tensor.matmul(
                            pA[:], a_t[:, kt * 128 : (kt + 1) * 128],
                            b_t[:, m0 : m0 + CH], start=True, stop=True,
                        )
                        r16 = rkeep.tile([128, CH], F16, tag="r16")
                        if RELU_DVE_MOD and kt % RELU_DVE_MOD == 0:
                            nc.vector.tensor_scalar_max(r16[:], pA[:], 0.0)
                        else:
                            nc.scalar.activation(r16[:], pA[:], AF.Relu)
                        rl.append(r16)
                        s16 = work.tile([128, CH], F16, tag="s16")
                        if SQ_ACT_MOD and kt % SQ_ACT_MOD == 0:
                            nc.scalar.activation(s16[:], r16[:], AF.Square)
                        else:
                            nc.vector.tensor_tensor(s16[:], r16[:], r16[:], OP.mult)
                        sq.append(s16)
                    if sq_prev is not None:
                        nc.tensor.matmul(
                            ss_ps[:], onc_t[:], sq_prev[kt][:],
                            start=(kt == 0), stop=(kt == NKT - 1),
                            skip_group_check=True,
                        )
                if c is not None:
                    sqs[c] = sq
                    relus[c] = rl

            def emit_recip(c):
                """ss row -> fp16 broadcast reciprocal-norm tile [128, CH]."""
                ss_ps = ss_rows.pop(c)
                ss_row = chain.tile([1, CH], F32, tag="ssrow")
                nc.scalar.activation(ss_row[:], ss_ps[:], AF.Copy)
                tpa_ps = psU.tile([128, 4], F32, tag="u")
                for j in range(4):
                    nc.tensor.transpose(
                        tpa_ps[:, j : j + 1],
                        ss_row[:, j * 128 : (j + 1) * 128], id_t[0:1, 0:1],
                    )
                jmm(4)
                ss_col = chain.tile([128, 4], F32, tag="sscol")
                nc.scalar.activation(ss_col[:], tpa_ps[:], AF.Copy)
                xx = chain.tile([128, 4], F32, tag="xx")
                nc.vector.tensor_scalar_add(xx[:], ss_col[:], EPS)
                s0 = chain.tile([128, 4], F32, tag="s0")
                nc.scalar.activation(s0[:], xx[:], AF.Sqrt)
                y0 = chain.tile([128, 4], F32, tag="y0")
                nc.vector.reciprocal(y0[:], s0[:])
                t0 = chain.tile([128, 4], F32, tag="t0")
                nc.vector.tensor_tensor(t0[:], y0[:], y0[:], OP.mult)
                nc.vector.tensor_tensor(t0[:], t0[:], xx[:], OP.mult)
                nc.vector.tensor_scalar(
                    out=t0[:], in0=t0[:], scalar1=-0.5, scalar2=1.5,
                    op0=OP.mult, op1=OP.add,
                )
                rc = chain.tile([128, 4], F32, tag="rc")
                nc.vector.tensor_tensor(rc[:], y0[:], t0[:], OP.mult)
                tpb_ps = psU.tile([1, CH], F32, tag="u")
                for j in range(4):
                    nc.tensor.transpose(
                        tpb_ps[:, j * 128 : (j + 1) * 128], rc[:, j : j + 1],
                        id_t[:],
                    )
                jmm(4)
                r_row = chain.tile([1, CH], F32R, tag="rrow")
                nc.scalar.activation(r_row[:], tpb_ps[:], AF.Copy)
                bc_ps = psU.tile([128, CH], F32, tag="u")
                nc.tensor.matmul(bc_ps[:], onr_t[:], r_row[:], start=True, stop=True)
                bc16 = chain.tile([128, CH], F16, tag="bc16")
                nc.scalar.activation(bc16[:], bc_ps[:], AF.Copy)
                return bc16

            def emit_B(c, bc16):
                """Scale kept relu tiles by recip, single casting DMA out."""
                m0 = c * CH
                big16 = bigp.tile([128, NKT * CH], F16, tag="big")
                rl = relus.pop(c)
                GRP = 6
                for g in range(NKT // GRP):
                    for kt in range(g * GRP, (g + 1) * GRP):
                        dstv = big16[:, kt * CH : (kt + 1) * CH]
                        if SC_GPS_MOD and kt % SC_GPS_MOD == 0:
                            nc.gpsimd.tensor_tensor(dstv, rl[kt][:], bc16[:], OP.mult)
                        else:
                            nc.vector.tensor_tensor(dstv, rl[kt][:], bc16[:], OP.mult)
                    dst = out_d[
                        g * GRP * 128 : (g + 1) * GRP * 128, m0 : m0 + CH
                    ].rearrange("(kt p) j -> p kt j", p=128)
                    src = big16[:, g * GRP * CH : (g + 1) * GRP * CH].rearrange(
                        "p (kt j) -> p kt j", j=CH
                    )
                    nc.gpsimd.dma_start(dst, src)
                    jmm(1)

            jmm(32)  # warm-up burst: flips HAM to 2.4 GHz before real work
            bcs = {}
            emit_A_and_ones(0, None)
            for i in range(1, NCH + 2):
                cA = i if i < NCH else None
                cO = i - 1 if i - 1 < NCH else None
                emit_A_and_ones(cA, cO)
                if cO is not None:
                    bcs[cO] = emit_recip(cO)
                if i - 2 >= 0:
                    emit_B(i - 2, bcs.pop(i - 2))
            junk_sb = chain.tile([128, CH], F32, tag="junksb")
            nc.scalar.activation(junk_sb[:], junk_ps[:], AF.Copy)
            nc.sync.dma_start(junk_d[:], junk_sb[:])

    nc.compile()
    return nc


def get_built():
    if "nc" not in _BUILD_CACHE:
        _BUILD_CACHE["nc"] = build()
    return _BUILD_CACHE["nc"]


def make_in_maps(x, y):
    ident = np.eye(128, dtype=np.float32)
    onescol = np.ones((128, 1), dtype=np.float16)
    onesrow = np.ones((1, 128), dtype=np.float32)
    in_maps = []
    for bi in range(B):
        a = np.ascontiguousarray(
            np.asarray(x)[bi].transpose(0, 2, 1).reshape(C, K)
        ).astype(np.float32)
        bm = np.ascontiguousarray(np.asarray(y)[bi].reshape(C, M)).astype(np.float32)
        in_maps.append(
            {"a": a, "b": bm, "ident": ident, "onescol": onescol, "onesrow": onesrow}
        )
    return in_maps


def run(x, y, trace=False):
    from concourse import bass_utils

    nc = get_built()
    in_maps = make_in_maps(x, y)
    res = bass_utils.run_bass_kernel_spmd(
        nc, in_maps, core_ids=list(range(B)), trace=trace
    )
    out = np.stack([res.results[bi]["out"].reshape(K, H, W) for bi in range(B)])
    return out, res


def kernel(x, y):
    out, _ = run(x, y, trace=False)
    return out

